# revision 3
# baseline (speedup 1.0000x reference)
"""MCANet channel-attention kernel for TRN2 (8 NeuronCores, data-parallel).

Reference math (the conv1x1+softmax branch in the module is dead code —
its result is deleted and never used):
    z[b,c]    = mean_{h,w} x[b,c,h,w]
    gate[b,c] = sigmoid(z[b,c] * w1d[c, center])       # center tap of the 1D conv
    out       = x * gate[:, :, None, None]

Per core: 2 batches of (512, 64*64) f32. Channels map to SBUF partitions
(4 blocks of 128), pixels to the free axis. One DMA in + reduce + sigmoid
+ broadcast-mul + one DMA out per (batch, channel-block) tile.
"""

import numpy as np

import concourse.tile as tile
from concourse import bacc, mybir
from concourse.bass_utils import run_bass_kernel_spmd

B, C, H, W = 16, 512, 64, 64
HW = H * W
K_CENTER = 2  # (5 - 1) // 2
N_CORES = 8
B_PER = B // N_CORES  # 2
P = 128
CBLK = C // P  # 4

_NC_CACHE = {}


def _build_nc(repeats=1, loop_n=None):
    nc = bacc.Bacc("TRN2", debug=False, target_bir_lowering=False,
                   num_devices=N_CORES)
    x_in = nc.dram_tensor("x", [B_PER, C, HW], mybir.dt.float32,
                          kind="ExternalInput").ap()
    wc_in = nc.dram_tensor("wc", [C], mybir.dt.float32,
                           kind="ExternalInput").ap()
    out = nc.dram_tensor("out", [B_PER, C, HW], mybir.dt.float32,
                         kind="ExternalOutput").ap()

    # DMA ring split: loads issue on the SP ring (nc.sync), stores on the
    # ACT ring (nc.scalar). HWDGE descriptors drain FIFO per ring and the
    # issuing sequencer blocks on unmet deps, so a store waiting for its
    # tile's multiply must not queue ahead of later loads — on separate
    # rings loads free-run while stores trail the ACT multiplies with zero
    # cross-engine sync (mul and store issue back-to-back on ACT).
    with tile.TileContext(nc) as tc:
        with (
            tc.tile_pool(name="xp", bufs=B_PER * CBLK) as xp,
            tc.tile_pool(name="sp", bufs=8 * max(1, repeats)) as sp,
            tc.tile_pool(name="wp", bufs=1) as wp,
        ):
            # wc laid out [partition, block]: element [p, t] = wc[t*128 + p]
            wt = wp.tile([P, CBLK], mybir.dt.float32)
            nc.sync.dma_start(wt[:], wc_in.rearrange("(t p) -> p t", p=P))

            def body():
                for b in range(B_PER):
                    for t in range(CBLK):
                        xt = xp.tile([P, HW], mybir.dt.float32)
                        nc.sync.dma_start(xt[:], x_in[b, t * P:(t + 1) * P, :])

                        s = sp.tile([P, 1], mybir.dt.float32)
                        nc.vector.reduce_sum(s[:], xt[:],
                                             axis=mybir.AxisListType.X)
                        # gate = sigmoid(sum * (w_center/HW)), w as scale AP
                        g = sp.tile([P, 1], mybir.dt.float32)
                        nc.scalar.activation(g[:], s[:],
                                             mybir.ActivationFunctionType.Sigmoid,
                                             scale=wt[:, t:t + 1])
                        # xt *= gate in place on ScalarE, store each chunk
                        # right behind its multiply on the same engine.
                        half = HW // 2
                        for h in range(2):
                            sl = slice(h * half, (h + 1) * half)
                            nc.scalar.mul(xt[:, sl], xt[:, sl], g[:])
                            nc.scalar.dma_start(
                                out[b, t * P:(t + 1) * P, sl], xt[:, sl])

            if loop_n is not None:
                with tc.For_i(0, loop_n):
                    body()
            else:
                for _ in range(repeats):
                    body()
    # Legalizes sync waits (≤1 per instruction, extras hoisted onto
    # EventSemaphore instructions) among other lowering passes.
    nc.compile()
    return nc


def _get_nc():
    if "nc" not in _NC_CACHE:
        _NC_CACHE["nc"] = _build_nc()
    return _NC_CACHE["nc"]


def _run(x, w1d, trace=False):
    x = np.ascontiguousarray(np.asarray(x, dtype=np.float32)).reshape(B, C, HW)
    # Fold the mean's 1/HW into the center-tap weight: HW is a power of two,
    # so w/HW is exact and sum*(w/HW) rounds identically to (sum/HW)*w.
    wc = np.ascontiguousarray(
        np.asarray(w1d, dtype=np.float32)[:, K_CENTER] / float(HW))
    nc = _get_nc()
    in_maps = [{"x": x[i * B_PER:(i + 1) * B_PER], "wc": wc}
               for i in range(N_CORES)]
    res = run_bass_kernel_spmd(nc, in_maps, list(range(N_CORES)), trace=trace)
    out = np.concatenate([res.results[i]["out"] for i in range(N_CORES)],
                         axis=0)
    return out.reshape(B, C, H, W), res.exec_time_ns


def kernel(x, w1x1=None, b1x1=None, w1d=None):
    out, _ = _run(x, w1d)
    return out

